# revision 30
# baseline (speedup 1.0000x reference)
"""Trainium2 Bass kernel for nn_ByteFormerWrapper (block_size=4096).

Math: reference computes img = byte2image_4k(x) (B,8,128,496) then
out = einsum('bchw,wo->bcho', img, W).

Key identity: img[b, c, p*8+s, i] = a_s[b, c, i+p] where
a_s[b, c, j] = (F >> (8-s)) & 255, F = 256*x[b,512c+j] + x[b,512c+j+1]
(next byte zero at j=511, per 512-byte sub-block). With norm(v) =
v*(2/255) - 1:
  out[b,c,p*8+s,o] = sum_j a_s[b,c,j] * Wsc_p[j,o] - S[o]
where Wsc_p is W*(2/255) zero-padded to 512 rows at offset p, S = W.sum(0).

The HW charges ~(cols * 0.42ns + 30ns) per matmul instruction with no
fp8/DoubleRow discount (measured), so the minimum-instruction mapping
wins: f16 operands, K=128 per instruction, 512 output columns -> 4
matmuls per (q, s-pair) PSUM tile, 128 total (~34us PE incl. loop
barrier).

Measured structure on HW: DMA queues share one bus (~0.3ns/B/partition
marginal, ~4-5us fixed latency per DMA chain); a body in isolation pays
an in-DMA head plus an evict+out-DMA tail around the ~28us PE stream.
Schedule decisions:
  1. Eight bodies are unrolled per For_i iteration: the semaphore-reset
     barrier amortizes 8x, each body's input DMA (dedicated scalar
     queue, finp bufs=3) prefetches during earlier bodies' compute, and
     its eviction/output tail overlaps later bodies' matmuls. No PE
     warmups needed - the PE never idles inside an iteration.
  2. Byte extraction stays ON DEVICE (2KB/part F upload; 8 DVE
     shift+mask + 8 casts ~8.5us): the U=8 unroll hides the DVE chain
     under the PE stream. (A host-precomputed 16KB/part f16-plane
     upload measured ~equal; the small-DMA variant wins slightly and
     loads 8x less.)
  3. Every PSUM tile is evicted by ACT and DVE in parallel (one 512-col
     half each), halving drain latency and PSUM recycle time.
Known residual (~7us/body over a const-fed matmul skeleton): PE
instructions carrying cross-engine/HWDGE sem waits run off the
hw-decode fast path. Ruled out as fixes: Pool/DVE bounce copies (Q7
slow / DVE serialization), dummy tail-reading ldweights (waits not
dominance-elided), single-engine eviction (tied), sem-update counts
(identical to fast skeleton).

Device schedule (per core, 32 batch rows => 256 (b,c) sub-blocks = bc):
  F[j_loc, k, bc] i16 in; per s-pair n: DVE shift+mask (i16) + copy-cast
  to f16 -> AT[j_loc, k, i, bc]. Weights ws[j, k, q, m] f16 loaded once;
  m = 64t + o, p = 2q + t. Per (q-pair, s-pair): 8 matmuls (2 q x 4 k
  chunks) -> PSUM [128, 1024] f32; ACT+DVE evict halves to f16; wide
  sync/gpsimd DMAs to OT[16, 64, 2048] = [p, o, s*256 + bc]; last chunk
  drains in 2-q pieces so the final pre-barrier DMA is small.
Host reassembles OT -> (256,8,128,64) f32.

Measured: 45210ns (U=1 baseline) -> 35396ns.
"""

import numpy as np

NCORES = 8
B = 256
B_LOC = B // NCORES  # 32 batch rows per core
SUB = 512
LAM_EXP_DEFAULT = 10  # f16 weights: Wsc * 2^10 ~ 0.76 max, comfortably normal
NWARM = 0

_CACHE = {}


def _build_program(repeat=1, lam_exp=LAM_EXP_DEFAULT, nwarm=NWARM):
    import concourse.mybir as mybir
    import concourse.tile as tile
    from concourse import bacc

    f32 = mybir.dt.float32
    f16 = mybir.dt.float16
    i16 = mybir.dt.int16
    Alu = mybir.AluOpType
    Ident = mybir.ActivationFunctionType.Identity

    nc = bacc.Bacc(None, target_bir_lowering=False, debug=False)

    with tile.TileContext(nc) as tc:
        with tc.tile_pool(name="dram", bufs=1, space="DRAM") as dram:
            f_d = dram.tile([128, 4, 256], i16, kind="ExternalInput", name="f", uniquify=False)
            ws_d = dram.tile([128, 4, 8, 128], f16, kind="ExternalInput", name="ws", uniquify=False)
            bias_d = dram.tile([128, 1], f32, kind="ExternalInput", name="bias", uniquify=False)
            ot_d = dram.tile([16, 64, 2048], f16, kind="ExternalOutput", name="ot", uniquify=False)
            ot_flat = ot_d.rearrange("p o n -> (p o) n")
            ot8 = ot_flat.rearrange("(qh pp) n -> pp qh n", qh=8)

            with (
                tc.tile_pool(name="const", bufs=1) as constp,
                tc.tile_pool(name="fin", bufs=3) as finp,
                tc.tile_pool(name="at", bufs=9) as atp,
                tc.tile_pool(name="sh", bufs=8) as shp,
                tc.tile_pool(name="mpsum", bufs=4, space="PSUM") as mpsum,
                tc.tile_pool(name="oev", bufs=6) as oevp,
            ):
                dW = constp.tile([128, 128], f16, name="dW")
                dA = constp.tile([128, 512], f16, name="dA")
                nc.vector.memset(dW, 1)
                nc.gpsimd.memset(dA, 1)
                bias_sb = constp.tile([128, 1], f32, name="bias_sb")
                ws_sb = constp.tile([128, 4, 8, 128], f16, name="ws_sb")
                for kh in range(2):
                    nc.scalar.dma_start(ws_sb[:, 2 * kh:2 * kh + 2], ws_d[:, 2 * kh:2 * kh + 2])
                nc.scalar.dma_start(bias_sb[:], bias_d[:])
                # preload the ACT Identity table before evictions need it
                warm = constp.tile([128, 1], f32, name="warm")
                nc.scalar.activation(warm[:], bias_sb[:], Ident, bias=bias_sb[:], scale=1.0)

                def body(warm=True):
                    # Host-plane DMA (16KB/partition) measured ~7us/body of
                    # PE slowdown: the DMA burst-writes into the same SBUF
                    # pool region the PE streams from (port conflicts). A
                    # 2KB/partition F upload + on-device DVE extraction
                    # avoids it: the DVE chain (8 shift+mask + 8 casts +
                    # 16 eviction halves ~18us) hides under the 28us PE
                    # stream thanks to the 8-body unroll.
                    F_all = finp.tile([128, 4, 256], i16, name="F", tag="F")
                    nc.scalar.dma_start(F_all[:], f_d[:])
                    ats = {}
                    for n in range(4):
                        AT = atp.tile([128, 4, 2, 256], f16, name="AT", tag="AT")
                        ats[n] = AT
                        for i in range(2):
                            sh = 8 - (2 * n + i)
                            sht = shp.tile([128, 4, 256], i16, name="sht", tag="sht")
                            nc.vector.tensor_scalar(
                                sht[:], F_all[:], sh, 255,
                                op0=Alu.logical_shift_right, op1=Alu.bitwise_and)
                            nc.vector.tensor_copy(AT[:, :, i, :], sht[:])
                    if warm and nwarm:
                        # dummy matmuls fill the PE during the A-load bubble and
                        # keep the p-state ramp warm across the loop barrier
                        psw = mpsum.tile([128, 1024], f32, name="psw", tag="ps")
                        for _ in range(nwarm):
                            nc.tensor.matmul(psw[:, 0:512], dW[:], dA[:], start=True, stop=True)

                    def evict(dst, ps, idx):
                        # PSUM already holds the final lam-domain value (bias is
                        # in-matmul): eviction is a pure f32 -> f16 copy; ACT
                        # and DVE each take one half so every tile drains in
                        # parallel (faster PSUM recycle, shorter tail)
                        half = 512
                        nc.scalar.activation(dst[:, 0:half], ps[:, 0:half], Ident, scale=1.0)
                        nc.vector.tensor_copy(dst[:, half:], ps[:, half:])

                    def mm_pair(n, j):
                        # two q's accumulate into one 2-bank PSUM tile, so a
                        # single eviction covers both
                        ps = mpsum.tile([128, 1024], f32, name="ps", tag="ps")
                        AT = ats[n]
                        for b in range(2):
                            q = 2 * j + b
                            for k in range(4):
                                nc.tensor.matmul(ps[:, 512 * b:512 * (b + 1)],
                                                 ws_sb[:, k, q], AT[:, k],
                                                 start=(k == 0), stop=(k == 3))
                        return ps

                    for n in range(4):
                        if n == 3:
                            # drain in 2-q pieces so the final DMA is small
                            for j in range(4):
                                ev = oevp.tile([128, 2, 512], f16, name="ev2", tag="ev2")
                                ps = mm_pair(n, j)
                                evict(ev.rearrange("p a c -> p (a c)"), ps, j)
                                eng = nc.sync if j % 2 == 0 else nc.gpsimd
                                eng.dma_start(ot8[:, 2 * j:2 * j + 2, 1536:2048], ev[:])
                        else:
                            for g in range(2):
                                ev = oevp.tile([128, 4, 512], f16, name="ev", tag="ev")
                                evf = ev.rearrange("p a c -> p (a c)")
                                for gi in range(2):
                                    ps = mm_pair(n, 2 * g + gi)
                                    evict(evf[:, 1024 * gi:1024 * (gi + 1)], ps, 2 * g + gi)
                                eng = nc.sync if g == 0 else nc.gpsimd
                                eng.dma_start(ot8[:, 4 * g:4 * g + 4, 512 * n:512 * (n + 1)], ev[:])

                if repeat == 1:
                    body()
                elif repeat < 0:  # unrolled (for cost-model experiments)
                    for _ in range(-repeat):
                        body()
                else:
                    # unroll two bodies per hardware-loop iteration: the
                    # For_i semaphore-reset barrier runs half as often, and
                    # body B's input DMAs / first matmuls overlap body A's
                    # eviction + output-DMA tail (finp/oev pools rotate)
                    done = False
                    for U in (64, 32, 16, 8, 4, 2):
                        if repeat % U == 0:
                            with tc.For_i(0, repeat // U):
                                for u in range(U):
                                    body(warm=(u == 0))
                            done = True
                            break
                    if not done:
                        with tc.For_i(0, repeat):
                            body()

    nc.finalize()
    return nc


def _quant_consts(W, lam_exp=LAM_EXP_DEFAULT):
    """Host-side f16 weight prep. Returns (ws, bias)."""
    W = np.asarray(W, dtype=np.float32)
    Wsc = W * (2.0 / 255.0)
    lam = 2.0 ** lam_exp
    W16 = (Wsc * lam).astype(np.float16)

    wpad = np.zeros((16, 512, 64), np.float16)
    for p in range(16):
        wpad[p, p:p + 496, :] = W16
    # bias folded into the matmul: host sets F[:,511] = 0xFFFF so a_s[511] is
    # 255 for every shift s, and the (always otherwise-zero) weight row 511
    # contributes 255 * (-S*lam/255) = -S*lam to every p copy.
    wpad[:, 511, :] = (-W.sum(0) * lam / 255.0).astype(np.float16)

    # ws[j_local, k, q, m]: m = 64t + o, p = 2q + t
    ws = np.zeros((128, 4, 8, 128), np.float16)
    for q in range(8):
        for t in range(2):
            wp = wpad[2 * q + t]  # [512, 64]
            for k in range(4):
                ws[:, k, q, 64 * t:64 * t + 64] = wp[128 * k:128 * (k + 1), :]

    bias = np.tile(-W.sum(0), 2).reshape(128, 1).astype(np.float32)
    return ws, bias


def _prep_f(x):
    """x (256, 4096) int -> per-core transposed F tensors [128, 4, 256] i16."""
    x = np.asarray(x)
    xb = x.astype(np.int64).reshape(B, 8, SUB)
    nxt = np.concatenate([xb[:, :, 1:], np.zeros((B, 8, 1), np.int64)], axis=2)
    F = (xb * 256 + nxt).astype(np.uint16)
    F[:, :, 511] = 0xFFFF  # sentinel: a_s[511] = 255 for all s (bias row)
    F = F.view(np.int16)  # [B, 8, 512]
    fs = []
    for r in range(NCORES):
        fj = F[r * B_LOC:(r + 1) * B_LOC].reshape(B_LOC * 8, SUB)  # [bc, j]
        ft = fj.T.reshape(4, 128, 256).transpose(1, 0, 2)          # [j_loc, k, bc]
        fs.append(np.ascontiguousarray(ft))
    return fs


def _prep_inputs(x, W):
    """Host-side prep: per-core int16 F tensors + replicated f16 weights."""
    ws, bias = _quant_consts(W)
    return [{"f": f, "ws": ws, "bias": bias} for f in _prep_f(x)]


def _assemble(results):
    """Per-core OT [16,64,2048] f16 -> (256,8,128,64) f32.

    OT column = s*256 + bc, bc = 8*b_loc + c.
    """
    outs = []
    for r in range(NCORES):
        ot = np.asarray(results[r]["ot"])
        o5 = ot.reshape(16, 64, 8, B_LOC, 8)          # [p, o, s, b_loc, c]
        outs.append(np.ascontiguousarray(
            o5.transpose(3, 4, 0, 2, 1)).reshape(B_LOC, 8, 128, 64))
    return np.concatenate(outs, axis=0).astype(np.float32) * np.float32(2.0 ** -LAM_EXP_DEFAULT)


def kernel(x, W):
    from concourse.bass_utils import run_bass_kernel_spmd

    key = ("nc", LAM_EXP_DEFAULT)
    if key not in _CACHE:
        _CACHE[key] = _build_program(repeat=1)
    nc = _CACHE[key]

    in_maps = _prep_inputs(x, W)
    res = run_bass_kernel_spmd(nc, in_maps, core_ids=list(range(NCORES)))
    return _assemble(res.results)
